# revision 32
# baseline (speedup 1.0000x reference)
"""Trainium2 Bass kernel for nn_CSABlock (dual spatial-attention gating).

Reference computation:
    sa_x  = sigmoid(conv3d(concat[max_c(x), mean_c(x)], w, k=7, pad=3))
    fix_out  = move * sa_fix + fix
    move_out = fix * sa_move + move

Sharding: 8 cores = (batch 2) x (D quarters of 20 planes). Each core gets a
26-plane input slab (3-voxel halo each side) per tensor in bf16 and produces
20 output planes in bf16; the host casts f32<->bf16 and shards/gathers.

Per-core pipeline (v5):
  - Stream D in chunks (3,4,4,4,4,4,3 planes). Load tile layout:
    partition=(d,hg32), free=(c16, hp3*w96) bf16 -> 576B contiguous lines.
  - Channel max tree on DVE, sum tree split DVE/GpSimd (mean's 1/16 is
    folded into the conv weights); final tree levels write fp8e4 stats.
  - Pooled stats staged into persistent P tiles [hin_pad128, stat2, dp28,
    wp102] fp8 via per-(d,hp) SBUF->SBUF reshape DMAs.
  - Conv: 49 fp8 DoubleRow matmuls per 4-plane output group; the two
    stats ride the k-tile dim; contraction over padded H with banded
    weights lhsT[hin, (kd,kw), c, hout] scaled by WS=256.
  - Sigmoid on ScalarE with scale=1/WS (PSUM -> SBUF bf16), reshaped to
    the data layout via 4 small DMAs.
  - Gating all-bf16: fix chain on DVE, move chain mostly on GpSimd.
  - DMA rings are isolated to avoid head-of-line blocking: loads flow on
    the sync ring with nothing ahead of them; P-stage/gate/store DMAs
    (which wait on compute semaphores) share the scalar ring in
    dependency order.
"""

import sys

import numpy as np

for _p in ("/opt/trn_rl_repo",):
    if _p not in sys.path:
        sys.path.insert(0, _p)

import ml_dtypes  # noqa: E402

B, C, D, H, W = 2, 16, 80, 96, 96
KK = 7
DSLAB = 28          # slab plane indexing (plane 0 and 27 never loaded)
OUTD = 20           # output planes per core
HG, HPW = 32, 3     # h = hg*3 + hp
WPAD = 102
NPAIR = KK * KK     # 49 DoubleRow matmuls per output group
NOC = 5             # output groups of G=4 planes
G = 4
NCORES = 8
WS = 256.0          # fp8 weight scale; undone in the sigmoid
CHUNKS = [(1, 3), (4, 4), (8, 4), (12, 4), (16, 4), (20, 4), (24, 3)]
LOADD = 26          # planes 1..26 inclusive

_prog_cache: dict = {}


def _build_banded(w: np.ndarray) -> np.ndarray:
    """w: [1,2,7,7,7] f32 -> lhsT [hin_pad 128, pair 49, c 2, hout 96] fp8e4.

    out[o,h,w'] = sum_{c,kd,kw} lhsT[hq, kd*7+kw, c, h] * P[hq, c, o+1+kd, w'+kw]
    with P[h_in+3, stat, dp, w_in+3] = pooled stats (0=max, 1=sum) and
    lhsT[h+kh, kd*7+kw, c, h] = w[c,kd,kh,kw] * WS * (1/16 for c=1).
    """
    A = np.zeros((128, NPAIR, 2, 96), np.float32)
    hh = np.arange(96)
    for c in range(2):
        scale = WS if c == 0 else WS / C
        for kd in range(KK):
            for kw in range(KK):
                pair = kd * KK + kw
                for kh in range(KK):
                    A[hh + kh, pair, c, hh] = w[0, c, kd, kh, kw] * scale
    return A.astype(ml_dtypes.float8_e4m3fn)


def _build_program():
    import concourse.bass as bass
    import concourse.bacc as bacc
    import concourse.tile as tile
    from concourse import mybir
    from contextlib import ExitStack

    bf16 = mybir.dt.bfloat16
    fp8 = mybir.dt.float8e4

    nc = bacc.Bacc("TRN2")
    fxs = nc.dram_tensor("fxs", [C, LOADD, H, W], bf16, kind="ExternalInput")
    mvs = nc.dram_tensor("mvs", [C, LOADD, H, W], bf16, kind="ExternalInput")
    wgf = nc.dram_tensor("wgf", [128, NPAIR, 2, 96], fp8, kind="ExternalInput")
    wgm = nc.dram_tensor("wgm", [128, NPAIR, 2, 96], fp8, kind="ExternalInput")
    fo = nc.dram_tensor("fo", [C, OUTD, H, W], bf16, kind="ExternalOutput")
    mo = nc.dram_tensor("mo", [C, OUTD, H, W], bf16, kind="ExternalOutput")

    with tile.TileContext(nc) as tc, ExitStack() as ctx:
        singles = ctx.enter_context(tc.tile_pool(name="singles", bufs=1))
        lpf = ctx.enter_context(tc.tile_pool(name="lpf", bufs=6))
        lpm = ctx.enter_context(tc.tile_pool(name="lpm", bufs=6))
        trpool = ctx.enter_context(tc.tile_pool(name="tr", bufs=4))
        pstage = ctx.enter_context(tc.tile_pool(name="pstage", bufs=5))
        tpool = ctx.enter_context(tc.tile_pool(name="tmp", bufs=6))
        gpool = ctx.enter_context(tc.tile_pool(name="gate", bufs=2))
        gtpool = ctx.enter_context(tc.tile_pool(name="gateT", bufs=3))
        psum = ctx.enter_context(tc.tile_pool(name="psum", bufs=4, space="PSUM"))

        WGF = singles.tile([128, NPAIR, 2, 96], fp8)
        WGM = singles.tile([128, NPAIR, 2, 96], fp8)

        # Persistent pooled tensors [hin_pad, stat(max,sum), dp, wp] fp8
        PF = singles.tile([128, 2, DSLAB, WPAD], fp8)
        PM = singles.tile([128, 2, DSLAB, WPAD], fp8)
        nc.gpsimd.memset(PF[:], 0.0)
        nc.gpsimd.memset(PM[:], 0.0)

        ltiles: dict = {}

        def load_chunk(ic: int):
            p0, nd = CHUNKS[ic]
            np_ = nd * HG
            for name, dram, lpool in (("f", fxs, lpf), ("m", mvs, lpm)):
                # partition order p = d*32 + hg, free (c, hp*w): one DMA,
                # 576B contiguous lines
                L = lpool.tile([128, C, HPW * W], bf16, tag="L" + name)
                src = dram[:, p0 - 1:p0 - 1 + nd, :, :].rearrange(
                    "c d (hg hp) w -> (d hg) c (hp w)", hg=HG, hp=HPW
                )
                nc.sync.dma_start(out=L[:np_], in_=src)
                ltiles[(name, ic)] = L

        def pool_chunk(ic: int):
            p0, nd = CHUNKS[ic]
            np_ = nd * HG  # active partitions
            for name, P in (("f", PF), ("m", PM)):
                L = ltiles[(name, ic)]

                # channel-reduction trees -> PS [np_, stat2, hp3, 96] fp8
                PS = pstage.tile([128, 2, HPW, W], fp8, tag="PS")
                Tmax = trpool.tile([128, 8, HPW * W], bf16, tag="Tmax")
                Tsum = trpool.tile([128, 8, HPW * W], bf16, tag="Tsum")
                Lv = L[:np_]
                PSv = PS[:np_].rearrange("p s hp w -> p s (hp w)")
                # max tree on DVE; sum tree L1 on DVE, rest on GpSimd
                nc.vector.tensor_max(Tmax[:np_], Lv[:, 0:8, :], Lv[:, 8:16, :])
                nc.vector.tensor_add(Tsum[:np_], Lv[:, 0:8, :], Lv[:, 8:16, :])
                nc.vector.tensor_max(Tmax[:np_, 0:4], Tmax[:np_, 0:4], Tmax[:np_, 4:8])
                nc.gpsimd.tensor_add(Tsum[:np_, 0:4], Tsum[:np_, 0:4], Tsum[:np_, 4:8])
                nc.vector.tensor_max(Tmax[:np_, 0:2], Tmax[:np_, 0:2], Tmax[:np_, 2:4])
                nc.gpsimd.tensor_add(Tsum[:np_, 0:2], Tsum[:np_, 0:2], Tsum[:np_, 2:4])
                nc.vector.tensor_max(PSv[:, 0], Tmax[:np_, 0], Tmax[:np_, 1])
                nc.gpsimd.tensor_add(PSv[:, 1], Tsum[:np_, 0], Tsum[:np_, 1])

                # stage into P: per (d, hp): src partitions d*32..d*32+31
                # (hg), free (stat, w); dst partitions 3+hp+3*hg (step 3).
                # Early chunks go fully on the scalar ring (the sync ring is
                # still draining the upfront loads, which would delay them);
                # later chunks alternate across both rings.
                for d in range(nd):
                    for hp in range(HPW):
                        if ic < 3:
                            eng = nc.scalar
                        else:
                            eng = nc.scalar if (d + hp) % 2 else nc.sync
                        eng.dma_start(
                            out=P[3 + hp:3 + hp + 94:3, :, p0 + d, 3:3 + W],
                            in_=PS[d * HG:(d + 1) * HG, :, hp, :],
                        )

        def conv_group(oc: int):
            o0 = G * oc
            gates = {}
            for name, P, WG in (("f", PF, WGF), ("m", PM, WGM)):
                acc = psum.tile([96, G, 96], mybir.dt.float32, tag="acc")
                for kd in range(KK):
                    dsl = slice(o0 + 1 + kd, o0 + 1 + kd + G)
                    for kw in range(KK):
                        nc.tensor.matmul(
                            acc[:],
                            WG[:, kd * KK + kw],
                            P[:, 0:2, dsl, kw:kw + 96],
                            start=(kd == 0 and kw == 0),
                            stop=(kd == KK - 1 and kw == KK - 1),
                            perf_mode=mybir.MatmulPerfMode.DoubleRow,
                        )
                gate = gpool.tile([96, G, 96], bf16, tag="gate")
                nc.scalar.activation(
                    out=gate[:], in_=acc[:],
                    func=mybir.ActivationFunctionType.Sigmoid,
                    scale=1.0 / WS,
                )
                # [96=h, (d,w)] -> gateT [128=(hg,d), (hp,w)]
                gateT = gtpool.tile([128, HPW, W], bf16, tag="gT")
                for d in range(G):
                    nc.scalar.dma_start(
                        out=gateT[d * HG:(d + 1) * HG, :, :], in_=gate[:, d, :]
                    )
                gates[name] = gateT
            return gates

        def elementwise(oc: int, gates):
            ic = oc + 1
            Lf, Lm = ltiles[("f", ic)], ltiles[("m", ic)]
            gf = (
                gates["f"][:].rearrange("p hp w -> p (hp w)").unsqueeze(1)
                .broadcast_to((128, 8, HPW * W))
            )
            gm = (
                gates["m"][:].rearrange("p hp w -> p (hp w)").unsqueeze(1)
                .broadcast_to((128, 8, HPW * W))
            )
            for q in range(2):
                cs = slice(q * 8, (q + 1) * 8)
                Tf = tpool.tile([128, 8, HPW * W], bf16, tag="T")
                Tm = tpool.tile([128, 8, HPW * W], bf16, tag="T")
                # fix chain on DVE: fo = move*gf + fix (broadcast operand
                # goes in slot 0; slot-1 broadcasts measured ~2x slower)
                nc.vector.tensor_mul(Tf[:], gf, Lm[:, cs, :])
                nc.vector.tensor_add(Tf[:], Tf[:], Lf[:, cs, :])
                # move chain entirely on GpSimd (DVE is the saturated
                # engine; same-engine chaining also avoids a cross-engine
                # semaphore)
                nc.gpsimd.tensor_mul(Tm[:], gm, Lf[:, cs, :])
                nc.gpsimd.tensor_add(Tm[:], Tm[:], Lm[:, cs, :])
                # stores ride the sync ring: all loads were issued up front,
                # so nothing queues behind these waits
                for T, dram in ((Tf, fo), (Tm, mo)):
                    dst = dram[cs, G * oc:G * oc + G, :, :].rearrange(
                        "c d (hg hp) w -> (d hg) c (hp w)", hg=HG, hp=HPW
                    )
                    nc.sync.dma_start(out=dst, in_=T[:])

        # software pipeline: all loads issued up front (L pools are deep
        # enough), keeping the sync ring free of compute-dependent waits.
        # Trees for the last two chunks are deferred into the oc loop so the
        # in-order DVE/GpSimd queues interleave them with early gating
        # instead of front-loading all trees and tail-loading all gating.
        for ic in range(6):
            load_chunk(ic)
            if ic == 2:
                # weight loads ride behind the first chunks: they head the
                # sync ring otherwise, delaying the first trees ~9us, but
                # aren't needed until conv(0)
                nc.sync.dma_start(out=WGF[:], in_=wgf[:])
                nc.sync.dma_start(out=WGM[:], in_=wgm[:])
        for ic in range(5):
            pool_chunk(ic)
        for oc in range(NOC):
            if oc == 0:
                load_chunk(6)
            gates = conv_group(oc)
            elementwise(oc, gates)
            if oc == 0:
                pool_chunk(5)
            elif oc == 1:
                pool_chunk(6)

    nc.compile()
    return nc


def _get_program():
    if "nc" not in _prog_cache:
        _prog_cache["nc"] = _build_program()
    return _prog_cache["nc"]


def _shard(fix, move, Af, Am):
    in_maps = []
    for core in range(NCORES):
        b, dq = core // 4, core % 4
        lo = 20 * dq - 3  # global index of slab plane 1
        s0, s1 = max(lo, 0), min(lo + LOADD, D)
        slab_f = np.zeros((C, LOADD, H, W), ml_dtypes.bfloat16)
        slab_m = np.zeros((C, LOADD, H, W), ml_dtypes.bfloat16)
        slab_f[:, s0 - lo:s1 - lo] = fix[b, :, s0:s1]
        slab_m[:, s0 - lo:s1 - lo] = move[b, :, s0:s1]
        in_maps.append({"fxs": slab_f, "mvs": slab_m, "wgf": Af, "wgm": Am})
    return in_maps


def kernel(fix, move, w_f2m, w_m2f, __trace=False):
    fix = np.asarray(fix, dtype=np.float32).astype(ml_dtypes.bfloat16)
    move = np.asarray(move, dtype=np.float32).astype(ml_dtypes.bfloat16)
    Af = _build_banded(np.asarray(w_f2m, dtype=np.float32))
    Am = _build_banded(np.asarray(w_m2f, dtype=np.float32))

    nc = _get_program()
    in_maps = _shard(fix, move, Af, Am)

    from concourse.bass_utils import run_bass_kernel_spmd

    res = run_bass_kernel_spmd(
        nc, in_maps, core_ids=list(range(NCORES)), trace=__trace
    )
    _prog_cache["last_results"] = res

    fix_out = np.empty((B, C, D, H, W), np.float32)
    move_out = np.empty((B, C, D, H, W), np.float32)
    for core in range(NCORES):
        b, dq = core // 4, core % 4
        fix_out[b, :, 20 * dq:20 * dq + 20] = res.results[core]["fo"].astype(
            np.float32
        )
        move_out[b, :, 20 * dq:20 * dq + 20] = res.results[core]["mo"].astype(
            np.float32
        )
    return fix_out, move_out
